# revision 25
# baseline (speedup 1.0000x reference)
"""Trainium2 Bass kernel for nn_NormConvTranspose2d.

Math: the reference applies, per (out-channel o, in-channel c), a
ConvTranspose2d(stride=2, k=3, pad=1, outpad=1) to input channel c with
kernel K[o,c], divides by the same convT applied to an all-ones image
(+eps), multiplies by weight[o,c], sums over c, adds bias.

With stride 2 / k 3, each output pixel (h', w') parity class is a fixed
1-4 tap correlation of the 48x48 input, and the "norm" denominator is a
per-(o,c) constant within each parity class (except at the last output
row/column).  So y/norm folds into effective channel-mixing matrices
W_tap[o,c] = weight*ktap/denom, and the whole module becomes channel-
mixing matmuls over column-shifted views of the input.

The host stacks x on 128 partitions as (x ; x shifted +48 = one row),
so every parity class needs only K=128 matmuls against column-shifted
views of ONE SBUF tile (one DMA, both halves at once).  With A=[oo|eo]
and B=[oe|ee] per chunk (8 output row-pairs, N=384), 3 matmuls total:
  A = [[Wi;Wc]|[Wf;0]] @ t[fb]  +  [[Wg;Wa]|[Wd;0]] @ t[fb+1]
  B = [[Wh;Wb]|[Wee;0]] @ t[fb]
Bias is applied by the PSUM->SBUF copy (per-partition bias operand);
the parity de-interleave happens on the host during the gather.
Edge passes (w'=95 col, h'=95 row, corner) are one small matmul each.

Sharding: 8 cores = 4 batches x 2 output-row halves (48 rows each).
No cross-core communication.
"""

import numpy as np

EPS = 1e-10
B, C, O, H, W = 4, 64, 64, 48, 48
HO = WO = 96
SLAB = 25          # input rows per core (24 + halo)
L = SLAB * 48      # 1200
LP = 1216          # padded free size of x tile
CHUNKS = (7, 6, 5, 6)   # row-pairs per chunk (big first: chunk 0 chews
                        # long enough that later chunks never stall on the
                        # input stream; 4 chunks, not 5: fewer tail
                        # LDWEIGHTS that can't hide under the tiny edge
                        # matmuls)
NMM = 480               # largest chunk's moving free size (PSUM tile width)
XSPLIT = 344            # first-slice columns of the x DMA (covers chunk 0;
                        # all slices stay >=512B/partition for DMA line rate)
WBW = 776              # wb width (bias col + 7 lhsT blocks)
WBSPLIT = 392          # main-chunk weights (bias+PA1+PA2+PB); splitting
                       # earlier delays the B matmuls and breaks the
                       # per-chunk A/B cadence
XOFF = WBSPLIT         # x's column offset inside the packed input tensor
XWW = WBSPLIT + LP + (WBW - WBSPLIT)   # 392 + 1216 + 384 = 1992
EOFF = WBSPLIT + LP    # edge-weight block offset (1608)
OUT_MAIN = 2 * 24 * 48     # 2304
OUT_W = OUT_MAIN + 73      # + rowEO(48) + colEO(24) + corner(1)

USE_BF16 = True    # bf16 datapath (f32 PSUM accumulate); else fp32r
USE_FP32R = True   # only relevant when USE_BF16 is False

_prog_cache = {}


def _build_program():
    import concourse.mybir as mybir
    import concourse.tile as tile
    from concourse import bacc

    f32 = mybir.dt.float32
    if USE_BF16:
        fmm = mybir.dt.bfloat16
    else:
        fmm = mybir.dt.float32r if USE_FP32R else f32
    Ident = mybir.ActivationFunctionType.Identity

    nc = bacc.Bacc("TRN2", target_bir_lowering=False, debug=False, num_devices=8)
    fio = fmm if USE_BF16 else f32
    # single packed input tensor [wb_main(392) | x(1200) | pad(16) | wb_edge]
    # so one DMA carries both of the first matmul's dependencies
    xw_d = nc.dram_tensor("xw", [128, XWW], fio, kind="ExternalInput").ap()
    out_d = nc.dram_tensor("out", [128, OUT_W], fio,
                           kind="ExternalOutput").ap()

    def D(ap):  # DRAM-side view matching the mm dtype tag
        return ap if USE_BF16 else ap.bitcast(fmm)

    with tile.TileContext(nc) as tc:
        with (
            tc.tile_pool(name="const", bufs=1) as cpool,
            tc.tile_pool(name="psum", bufs=3, space="PSUM") as ppool,
            tc.tile_pool(name="psum1", bufs=1, space="PSUM") as ppool1,
        ):
            xw = cpool.tile([128, XWW], fmm)
            och = cpool.tile([128, OUT_W], fio)
            warm = cpool.tile([64, 1], f32)

            nc.vector.memset(warm[:], 0.0)

            # input in 3 DMAs, ALL on the sync queue: the 16 DMA engines
            # process one queue's descriptors FIFO, so the slices land in
            # exact consumption order with no bandwidth competition — the
            # first slice (weights + chunk-0 x) gets the full line rate.
            # NOTE: slices must keep >=~1KB per partition row — small-row
            # DMAs run at a fraction of line rate (measured).
            S1 = XOFF + XSPLIT
            S2 = XOFF + 868
            nc.sync.dma_start(xw[:, 0:S1], D(xw_d[:, 0:S1]))
            nc.sync.dma_start(xw[:, S1:S2], D(xw_d[:, S1:S2]))
            nc.sync.dma_start(xw[:, S2:XWW], D(xw_d[:, S2:XWW]))
            # warm the Scalar activation table early
            nc.scalar.activation(warm[:], warm[:], Ident, bias=0.0)

            if USE_BF16:
                # upcast the per-partition bias column once for the copies
                btile = cpool.tile([128, 1], f32)
                nc.vector.tensor_copy(btile[:], xw[:, 0:1])
                bt = btile[:]
            else:
                bt = xw[:, 0:1].bitcast(f32)

            def Wp(off, m=128):
                return xw[:, off : off + m]

            PA1, PA2, PB = Wp(8), Wp(136), Wp(264)
            R, R2 = Wp(EOFF), Wp(EOFF + 128, 64)
            Cm, CR = Wp(EOFF + 192), Wp(EOFF + 320, 64)

            def xs(fb, n=NMM):
                return xw[:, XOFF + fb : XOFF + fb + n]

            # stage the w'=95 input column for the edge matmuls (xcol
            # bottom = x[r+1, 47] via the +48-shifted stacked half, same
            # partitions).  On gpsimd: it's gated on the last input slice,
            # and on vector it would block the chunk A-adds (in-order queue)
            xvt = xw[0:64, XOFF : XOFF + L].rearrange("p (r q) -> p r q",
                                                      q=48)
            xvb = xw[64:128, XOFF : XOFF + L].rearrange("p (r q) -> p r q",
                                                        q=48)
            xcol = cpool.tile([128, 32], fmm)
            nc.gpsimd.tensor_copy(xcol[0:64, 0:SLAB], xvt[:, :, 47])
            nc.gpsimd.tensor_copy(xcol[64:128, 0:24], xvb[:, 0:24, 47])

            def chunk(ci):
                r0 = sum(CHUNKS[:ci])
                n = CHUNKS[ci] * 48
                fb, base = 48 * r0, 96 * r0
                A = ppool.tile([128, NMM], f32, tag="A")
                nc.tensor.matmul(A[:, 0:n], PA1, xs(fb, n), start=True,
                                 stop=False)
                nc.tensor.matmul(A[:, 0:n], PA2, xs(fb + 1, n), start=False,
                                 stop=True)
                Bp = ppool.tile([128, NMM], f32, tag="B")
                nc.tensor.matmul(Bp[:, 0:n], PB, xs(fb, n), start=True,
                                 stop=True)
                nc.vector.tensor_scalar_add(och[:, base : base + n],
                                            A[:, 0:n], bt)
                nc.scalar.activation(och[:, base + n : base + 2 * n],
                                     Bp[:, 0:n], Ident, bias=bt)

            chunk(0)
            chunk(1)
            # out DMAs grouped (fewer trips through the shared HWDGE
            # descriptor unit, ~625ns each); all on the idle sync queue,
            # staggered so the engine pool streams them back-to-back
            G0 = 96 * sum(CHUNKS[:2])
            nc.sync.dma_start(out_d[:, 0:G0], och[:, 0:G0])
            chunk(2)
            G1 = 96 * sum(CHUNKS[:3])
            nc.sync.dma_start(out_d[:, G0:G1], och[:, G0:G1])
            chunk(3)

            # --- edge passes (results appended at och[:, 2304:]) ---
            # scheduled last: they read the tail input slice, and their
            # small PSUM->SBUF adds are the natural final och write
            # last output row (h'=95; used by the host only for half=1)
            Er = ppool1.tile([128, 48], f32, tag="Er")
            nc.tensor.matmul(Er[:], R, xs(1104, 48), start=True, stop=False)
            nc.tensor.matmul(Er[0:64, :], R2, xs(1105, 48), start=False,
                             stop=True)
            # last output column (w'=95), plus the corner at col 24
            Ec = ppool1.tile([128, 32], f32, tag="Ec")
            nc.tensor.matmul(Ec[:, 0:24], Cm, xcol[:, 0:24], start=True,
                             stop=True)
            # N=1 is illegal for fp32r; bf16 runs it padded to N=2 and the
            # fp32r path falls back to a plain-f32 matmul
            if USE_BF16:
                nc.tensor.matmul(Ec[0:64, 24:26], CR, xs(1151, 2),
                                 start=True, stop=True)
            else:
                nc.tensor.matmul(Ec[0:64, 24:25], CR.bitcast(f32),
                                 xs(1151, 1).bitcast(f32), start=True,
                                 stop=True)
            nc.vector.tensor_scalar_add(och[:, 2304:2352], Er[:], bt)
            nc.vector.tensor_scalar_add(och[:, 2352:2377], Ec[:, 0:25], bt)

            # final group: chunks 3+4 + the edge columns
            nc.sync.dma_start(out_d[:, G1:OUT_W], och[:, G1:OUT_W])

    nc.compile()
    return nc


def _io_dtype():
    if USE_BF16:
        import ml_dtypes
        return ml_dtypes.bfloat16
    return np.float32


def _round_fp32r(a):
    """Quantize to the PE grid: bf16, or 11-mantissa-bit FP32R."""
    if USE_BF16:
        return np.ascontiguousarray(a, np.float32).astype(_io_dtype())
    if not USE_FP32R:
        return np.ascontiguousarray(a, np.float32)
    u = np.ascontiguousarray(a, np.float32).view(np.uint32)
    r = (u + np.uint32(0x7FF) + ((u >> np.uint32(12)) & np.uint32(1))) \
        & np.uint32(0xFFFFF000)
    return r.view(np.float32)


def _eff_weights(weight, kernels, bias):
    """Host-side constant folding: effective channel-mix matrices packed as
    K=128 lhsT blocks [128, 712] (col 0 = per-partition bias for the copy
    ops; the K rows match the stacked (x ; x+1) moving tile)."""
    w = weight.astype(np.float64)
    k = kernels.astype(np.float64)
    k00, k01, k02 = k[:, :, 0, 0], k[:, :, 0, 1], k[:, :, 0, 2]
    k10, k11, k12 = k[:, :, 1, 0], k[:, :, 1, 1], k[:, :, 1, 2]
    k20, k21, k22 = k[:, :, 2, 0], k[:, :, 2, 1], k[:, :, 2, 2]

    den_oo = k22 + k20 + k02 + k00 + EPS
    M = dict(
        Wee=w * k11 / (k11 + EPS),
        Wf=w * k12 / (k12 + k10 + EPS), Wd=w * k10 / (k12 + k10 + EPS),
        Wh=w * k21 / (k21 + k01 + EPS), Wb=w * k01 / (k21 + k01 + EPS),
        Wi=w * k22 / den_oo, Wg=w * k20 / den_oo,
        Wc=w * k02 / den_oo, Wa=w * k00 / den_oo,
        Ef=w * k12 / (k12 + EPS),
        Ei=w * k22 / (k22 + k02 + EPS), Ec=w * k02 / (k22 + k02 + EPS),
        Rh=w * k21 / (k21 + EPS),
        Ri=w * k22 / (k22 + k20 + EPS), Rg=w * k20 / (k22 + k20 + EPS),
        Ci=w * k22 / (k22 + EPS),
    )
    T = {n: m.T for n, m in M.items()}  # lhsT orientation [c, o]
    Z = np.zeros((64, 64))

    wbm = np.zeros((128, WBW))
    wbm[0:64, 0] = bias.astype(np.float64)
    wbm[64:128, 0] = bias.astype(np.float64)
    blocks = [
        (8, [[T["Wi"], T["Wf"]], [T["Wc"], Z]]),          # PA1 @ t[fb]
        (136, [[T["Wg"], T["Wd"]], [T["Wa"], Z]]),        # PA2 @ t[fb+1]
        (264, [[T["Wh"], T["Wee"]], [T["Wb"], Z]]),       # PB  @ t[fb]
        (392, [[T["Ri"], T["Rh"]], [Z, Z]]),              # R   @ t[1104]
        (520, [[T["Rg"]], [Z]]),                          # R2  @ t[1105]
        (584, [[T["Ei"], T["Ef"]], [T["Ec"], Z]]),        # Cm  @ xcol
        (712, [[T["Ci"]], [Z]]),                          # CR  @ t[1151]
    ]
    for off, blk in blocks:
        b = np.block(blk)
        wbm[:, off : off + b.shape[1]] = b
    return _round_fp32r(wbm)


def _make_in_maps(input, weight, kernels, bias):
    dt = _io_dtype()
    wbm = _eff_weights(weight, kernels, bias)
    x = np.asarray(input, np.float32).astype(dt)
    in_maps = []
    for core in range(8):
        b, half = core // 2, core % 2
        slab = np.zeros((C, SLAB, 48), dt)
        if half == 0:
            slab[:, :, :] = x[b, :, 0:25, :]
        else:
            slab[:, 0:24, :] = x[b, :, 24:48, :]
        flat = slab.reshape(C, L)
        xw = np.zeros((128, XWW), dt)
        xw[:, 0:WBSPLIT] = wbm[:, 0:WBSPLIT]
        xw[0:64, XOFF : XOFF + L] = flat
        xw[64:128, XOFF : XOFF + L - 48] = flat[:, 48:]
        xw[:, EOFF:XWW] = wbm[:, WBSPLIT:WBW]
        in_maps.append({"xw": xw})
    return in_maps


def kernel(input, weight, kernels, bias):
    from concourse.bass_utils import run_bass_kernel_spmd

    input = np.asarray(input)
    weight = np.asarray(weight)
    kernels = np.asarray(kernels)
    bias = np.asarray(bias)

    if "nc" not in _prog_cache:
        _prog_cache["nc"] = _build_program()
    nc = _prog_cache["nc"]

    in_maps = _make_in_maps(input, weight, kernels, bias)
    res = run_bass_kernel_spmd(nc, in_maps, core_ids=list(range(8)))

    out = np.empty((B, O, HO, WO), np.float32)
    blk = np.empty((O, 48, WO), np.float32)
    for core in range(8):
        b, half = core // 2, core % 2
        r = np.asarray(res.results[core]["out"]).astype(np.float32)
        edge = r[:, OUT_MAIN:]
        # per chunk: [A(n) | B(n)] with A=[oo|eo], B=[oe|ee]
        r0 = 0
        for nc_ in CHUNKS:
            base, n = 96 * r0, 48 * nc_
            Ab = r[:, base : base + n].reshape(128, nc_, 48)
            Bb = r[:, base + n : base + 2 * n].reshape(128, nc_, 48)
            rows = slice(2 * r0, 2 * (r0 + nc_))
            blk[:, rows, :][:, 1::2, 1::2] = Ab[0:64]
            blk[:, rows, :][:, 0::2, 1::2] = Ab[64:128]
            blk[:, rows, :][:, 1::2, 0::2] = Bb[0:64]
            blk[:, rows, :][:, 0::2, 0::2] = Bb[64:128]
            r0 += nc_
        # w'=95 column fixup: Ec=[colO|colE] at cols 48:72
        blk[:, 1::2, 95] = edge[0:64, 48:72]
        blk[:, 0::2, 95] = edge[64:128, 48:72]
        if half == 1:
            # h'=95 row fixup: Er=[rowO|rowE] at cols 0:48, corner at 72
            blk[:, 47, 1::2] = edge[0:64, 0:48]
            blk[:, 47, 0::2] = edge[64:128, 0:48]
            blk[:, 47, 95] = edge[0:64, 72]
        out[b, :, half * 48 : (half + 1) * 48, :] = blk
    return out



# revision 26
# speedup vs baseline: 1.0271x; 1.0271x over previous
"""Trainium2 Bass kernel for nn_NormConvTranspose2d.

Math: the reference applies, per (out-channel o, in-channel c), a
ConvTranspose2d(stride=2, k=3, pad=1, outpad=1) to input channel c with
kernel K[o,c], divides by the same convT applied to an all-ones image
(+eps), multiplies by weight[o,c], sums over c, adds bias.

With stride 2 / k 3, each output pixel (h', w') parity class is a fixed
1-4 tap correlation of the 48x48 input, and the "norm" denominator is a
per-(o,c) constant within each parity class (except at the last output
row/column).  So y/norm folds into effective channel-mixing matrices
W_tap[o,c] = weight*ktap/denom, and the whole module becomes channel-
mixing matmuls over column-shifted views of the input.

The host stacks x on 128 partitions as (x ; x shifted +48 = one row),
so every parity class needs only K=128 matmuls against column-shifted
views of ONE SBUF tile (one DMA, both halves at once).  With A=[oo|eo]
and B=[oe|ee] per chunk (8 output row-pairs, N=384), 3 matmuls total:
  A = [[Wi;Wc]|[Wf;0]] @ t[fb]  +  [[Wg;Wa]|[Wd;0]] @ t[fb+1]
  B = [[Wh;Wb]|[Wee;0]] @ t[fb]
Bias is applied by the PSUM->SBUF copy (per-partition bias operand);
the parity de-interleave happens on the host during the gather.
Edge passes (w'=95 col, h'=95 row, corner) are one small matmul each.

Sharding: 8 cores = 4 batches x 2 output-row halves (48 rows each).
No cross-core communication.
"""

import numpy as np

EPS = 1e-10
B, C, O, H, W = 4, 64, 64, 48, 48
HO = WO = 96
SLAB = 25          # input rows per core (24 + halo)
L = SLAB * 48      # 1200
LP = 1216          # padded free size of x tile
CHUNKS = (7, 6, 5, 6)   # row-pairs per chunk (big first: chunk 0 chews
                        # long enough that later chunks never stall on the
                        # input stream; 4 chunks, not 5: fewer tail
                        # LDWEIGHTS that can't hide under the tiny edge
                        # matmuls)
NMM = 480               # largest chunk's moving free size (PSUM tile width)
XSPLIT = 344            # first-slice columns of the x DMA (covers chunk 0;
                        # all slices stay >=512B/partition for DMA line rate)
WBW = 776              # wb width (bias col + 7 lhsT blocks)
WBSPLIT = 392          # main-chunk weights (bias+PA1+PA2+PB); splitting
                       # earlier delays the B matmuls and breaks the
                       # per-chunk A/B cadence
XOFF = WBSPLIT         # x's column offset inside the packed input tensor
XWW = WBSPLIT + LP + (WBW - WBSPLIT)   # 392 + 1216 + 384 = 1992
EOFF = WBSPLIT + LP    # edge-weight block offset (1608)
OUT_MAIN = 2 * 24 * 48     # 2304
OUT_W = OUT_MAIN + 73      # + rowEO(48) + colEO(24) + corner(1)

USE_BF16 = True    # bf16 datapath (f32 PSUM accumulate); else fp32r
USE_FP32R = True   # only relevant when USE_BF16 is False

_prog_cache = {}


def _build_program():
    import concourse.mybir as mybir
    import concourse.tile as tile
    from concourse import bacc

    f32 = mybir.dt.float32
    if USE_BF16:
        fmm = mybir.dt.bfloat16
    else:
        fmm = mybir.dt.float32r if USE_FP32R else f32
    Ident = mybir.ActivationFunctionType.Identity

    nc = bacc.Bacc("TRN2", target_bir_lowering=False, debug=False, num_devices=8)
    fio = fmm if USE_BF16 else f32
    # single packed input tensor [wb_main(392) | x(1200) | pad(16) | wb_edge]
    # so one DMA carries both of the first matmul's dependencies
    xw_d = nc.dram_tensor("xw", [128, XWW], fio, kind="ExternalInput").ap()
    out_d = nc.dram_tensor("out", [128, OUT_W], fio,
                           kind="ExternalOutput").ap()

    def D(ap):  # DRAM-side view matching the mm dtype tag
        return ap if USE_BF16 else ap.bitcast(fmm)

    with tile.TileContext(nc) as tc:
        with (
            tc.tile_pool(name="const", bufs=1) as cpool,
            tc.tile_pool(name="psum", bufs=3, space="PSUM") as ppool,
            tc.tile_pool(name="psum1", bufs=1, space="PSUM") as ppool1,
        ):
            xw = cpool.tile([128, XWW], fmm)
            och = cpool.tile([128, OUT_W], fio)
            warm = cpool.tile([64, 1], f32)

            nc.vector.memset(warm[:], 0.0)

            # input in 3 DMAs, ALL on the sync queue: the 16 DMA engines
            # process one queue's descriptors FIFO, so the slices land in
            # exact consumption order with no bandwidth competition — the
            # first slice (weights + chunk-0 x) gets the full line rate.
            # NOTE: slices must keep >=~1KB per partition row — small-row
            # DMAs run at a fraction of line rate (measured).
            S1 = XOFF + XSPLIT
            S2 = XOFF + 868
            nc.sync.dma_start(xw[:, 0:S1], D(xw_d[:, 0:S1]))
            nc.sync.dma_start(xw[:, S1:S2], D(xw_d[:, S1:S2]))
            nc.sync.dma_start(xw[:, S2:XWW], D(xw_d[:, S2:XWW]))
            # warm the Scalar activation table early
            nc.scalar.activation(warm[:], warm[:], Ident, bias=0.0)

            if USE_BF16:
                # upcast the per-partition bias column once for the copies
                btile = cpool.tile([128, 1], f32)
                nc.vector.tensor_copy(btile[:], xw[:, 0:1])
                bt = btile[:]
            else:
                bt = xw[:, 0:1].bitcast(f32)

            def Wp(off, m=128):
                return xw[:, off : off + m]

            PA1, PA2, PB = Wp(8), Wp(136), Wp(264)
            R, R2 = Wp(EOFF), Wp(EOFF + 128, 64)
            Cm, CR = Wp(EOFF + 192), Wp(EOFF + 320, 64)

            def xs(fb, n=NMM):
                return xw[:, XOFF + fb : XOFF + fb + n]

            # stage the w'=95 input column for the edge matmuls (xcol
            # bottom = x[r+1, 47] via the +48-shifted stacked half, same
            # partitions).  On gpsimd: it's gated on the last input slice,
            # and on vector it would block the chunk A-adds (in-order queue)
            xvt = xw[0:64, XOFF : XOFF + L].rearrange("p (r q) -> p r q",
                                                      q=48)
            xvb = xw[64:128, XOFF : XOFF + L].rearrange("p (r q) -> p r q",
                                                        q=48)
            xcol = cpool.tile([128, 32], fmm)
            nc.gpsimd.tensor_copy(xcol[0:64, 0:SLAB], xvt[:, :, 47])
            nc.gpsimd.tensor_copy(xcol[64:128, 0:24], xvb[:, 0:24, 47])

            def chunk(ci):
                r0 = sum(CHUNKS[:ci])
                n = CHUNKS[ci] * 48
                fb, base = 48 * r0, 96 * r0
                A = ppool.tile([128, NMM], f32, tag="A")
                nc.tensor.matmul(A[:, 0:n], PA1, xs(fb, n), start=True,
                                 stop=False)
                nc.tensor.matmul(A[:, 0:n], PA2, xs(fb + 1, n), start=False,
                                 stop=True)
                Bp = ppool.tile([128, NMM], f32, tag="B")
                nc.tensor.matmul(Bp[:, 0:n], PB, xs(fb, n), start=True,
                                 stop=True)
                nc.vector.tensor_scalar_add(och[:, base : base + n],
                                            A[:, 0:n], bt)
                nc.scalar.activation(och[:, base + n : base + 2 * n],
                                     Bp[:, 0:n], Ident, bias=bt)

            chunk(0)
            chunk(1)
            # out DMAs grouped (fewer trips through the shared HWDGE
            # descriptor unit, ~625ns each); all on the idle sync queue,
            # staggered so the engine pool streams them back-to-back
            G0 = 96 * sum(CHUNKS[:2])
            nc.sync.dma_start(out_d[:, 0:G0], och[:, 0:G0])
            chunk(2)
            G1 = 96 * sum(CHUNKS[:3])
            nc.sync.dma_start(out_d[:, G0:G1], och[:, G0:G1])
            chunk(3)

            # --- edge passes (results appended at och[:, 2304:]) ---
            # scheduled last: they read the tail input slice.  All edge
            # results land in ONE PSUM bank so a single bias-add produces
            # the final och region (shortest possible last och write).
            # E[:,0:48] = last output row (h'=95; host uses it for half=1),
            # E[:,48:72] = last output column (w'=95), E[:,72] = corner.
            E = ppool1.tile([128, 80], f32, tag="E")
            nc.tensor.matmul(E[:, 0:48], R, xs(1104, 48), start=True,
                             stop=False)
            nc.tensor.matmul(E[0:64, 0:48], R2, xs(1105, 48), start=False,
                             stop=True)
            nc.tensor.matmul(E[:, 48:72], Cm, xcol[:, 0:24], start=True,
                             stop=True)
            # N=1 is illegal for fp32r; bf16 runs it padded to N=2 and the
            # fp32r path falls back to a plain-f32 matmul
            if USE_BF16:
                nc.tensor.matmul(E[0:64, 72:74], CR, xs(1151, 2),
                                 start=True, stop=True)
            else:
                nc.tensor.matmul(E[0:64, 72:73], CR.bitcast(f32),
                                 xs(1151, 1).bitcast(f32), start=True,
                                 stop=True)
            nc.vector.tensor_scalar_add(och[:, 2304:2377], E[:, 0:73], bt)

            # final group: last chunk + the edge columns
            nc.sync.dma_start(out_d[:, G1:OUT_W], och[:, G1:OUT_W])

    nc.compile()
    return nc


def _io_dtype():
    if USE_BF16:
        import ml_dtypes
        return ml_dtypes.bfloat16
    return np.float32


def _round_fp32r(a):
    """Quantize to the PE grid: bf16, or 11-mantissa-bit FP32R."""
    if USE_BF16:
        return np.ascontiguousarray(a, np.float32).astype(_io_dtype())
    if not USE_FP32R:
        return np.ascontiguousarray(a, np.float32)
    u = np.ascontiguousarray(a, np.float32).view(np.uint32)
    r = (u + np.uint32(0x7FF) + ((u >> np.uint32(12)) & np.uint32(1))) \
        & np.uint32(0xFFFFF000)
    return r.view(np.float32)


def _eff_weights(weight, kernels, bias):
    """Host-side constant folding: effective channel-mix matrices packed as
    K=128 lhsT blocks [128, 712] (col 0 = per-partition bias for the copy
    ops; the K rows match the stacked (x ; x+1) moving tile)."""
    w = weight.astype(np.float64)
    k = kernels.astype(np.float64)
    k00, k01, k02 = k[:, :, 0, 0], k[:, :, 0, 1], k[:, :, 0, 2]
    k10, k11, k12 = k[:, :, 1, 0], k[:, :, 1, 1], k[:, :, 1, 2]
    k20, k21, k22 = k[:, :, 2, 0], k[:, :, 2, 1], k[:, :, 2, 2]

    den_oo = k22 + k20 + k02 + k00 + EPS
    M = dict(
        Wee=w * k11 / (k11 + EPS),
        Wf=w * k12 / (k12 + k10 + EPS), Wd=w * k10 / (k12 + k10 + EPS),
        Wh=w * k21 / (k21 + k01 + EPS), Wb=w * k01 / (k21 + k01 + EPS),
        Wi=w * k22 / den_oo, Wg=w * k20 / den_oo,
        Wc=w * k02 / den_oo, Wa=w * k00 / den_oo,
        Ef=w * k12 / (k12 + EPS),
        Ei=w * k22 / (k22 + k02 + EPS), Ec=w * k02 / (k22 + k02 + EPS),
        Rh=w * k21 / (k21 + EPS),
        Ri=w * k22 / (k22 + k20 + EPS), Rg=w * k20 / (k22 + k20 + EPS),
        Ci=w * k22 / (k22 + EPS),
    )
    T = {n: m.T for n, m in M.items()}  # lhsT orientation [c, o]
    Z = np.zeros((64, 64))

    wbm = np.zeros((128, WBW))
    wbm[0:64, 0] = bias.astype(np.float64)
    wbm[64:128, 0] = bias.astype(np.float64)
    blocks = [
        (8, [[T["Wi"], T["Wf"]], [T["Wc"], Z]]),          # PA1 @ t[fb]
        (136, [[T["Wg"], T["Wd"]], [T["Wa"], Z]]),        # PA2 @ t[fb+1]
        (264, [[T["Wh"], T["Wee"]], [T["Wb"], Z]]),       # PB  @ t[fb]
        (392, [[T["Ri"], T["Rh"]], [Z, Z]]),              # R   @ t[1104]
        (520, [[T["Rg"]], [Z]]),                          # R2  @ t[1105]
        (584, [[T["Ei"], T["Ef"]], [T["Ec"], Z]]),        # Cm  @ xcol
        (712, [[T["Ci"]], [Z]]),                          # CR  @ t[1151]
    ]
    for off, blk in blocks:
        b = np.block(blk)
        wbm[:, off : off + b.shape[1]] = b
    return _round_fp32r(wbm)


def _make_in_maps(input, weight, kernels, bias):
    dt = _io_dtype()
    wbm = _eff_weights(weight, kernels, bias)
    x = np.asarray(input, np.float32).astype(dt)
    in_maps = []
    for core in range(8):
        b, half = core // 2, core % 2
        slab = np.zeros((C, SLAB, 48), dt)
        if half == 0:
            slab[:, :, :] = x[b, :, 0:25, :]
        else:
            slab[:, 0:24, :] = x[b, :, 24:48, :]
        flat = slab.reshape(C, L)
        xw = np.zeros((128, XWW), dt)
        xw[:, 0:WBSPLIT] = wbm[:, 0:WBSPLIT]
        xw[0:64, XOFF : XOFF + L] = flat
        xw[64:128, XOFF : XOFF + L - 48] = flat[:, 48:]
        xw[:, EOFF:XWW] = wbm[:, WBSPLIT:WBW]
        in_maps.append({"xw": xw})
    return in_maps


def kernel(input, weight, kernels, bias):
    from concourse.bass_utils import run_bass_kernel_spmd

    input = np.asarray(input)
    weight = np.asarray(weight)
    kernels = np.asarray(kernels)
    bias = np.asarray(bias)

    if "nc" not in _prog_cache:
        _prog_cache["nc"] = _build_program()
    nc = _prog_cache["nc"]

    in_maps = _make_in_maps(input, weight, kernels, bias)
    res = run_bass_kernel_spmd(nc, in_maps, core_ids=list(range(8)))

    out = np.empty((B, O, HO, WO), np.float32)
    blk = np.empty((O, 48, WO), np.float32)
    for core in range(8):
        b, half = core // 2, core % 2
        r = np.asarray(res.results[core]["out"]).astype(np.float32)
        edge = r[:, OUT_MAIN:]
        # per chunk: [A(n) | B(n)] with A=[oo|eo], B=[oe|ee]
        r0 = 0
        for nc_ in CHUNKS:
            base, n = 96 * r0, 48 * nc_
            Ab = r[:, base : base + n].reshape(128, nc_, 48)
            Bb = r[:, base + n : base + 2 * n].reshape(128, nc_, 48)
            rows = slice(2 * r0, 2 * (r0 + nc_))
            blk[:, rows, :][:, 1::2, 1::2] = Ab[0:64]
            blk[:, rows, :][:, 0::2, 1::2] = Ab[64:128]
            blk[:, rows, :][:, 1::2, 0::2] = Bb[0:64]
            blk[:, rows, :][:, 0::2, 0::2] = Bb[64:128]
            r0 += nc_
        # w'=95 column fixup: Ec=[colO|colE] at cols 48:72
        blk[:, 1::2, 95] = edge[0:64, 48:72]
        blk[:, 0::2, 95] = edge[64:128, 48:72]
        if half == 1:
            # h'=95 row fixup: Er=[rowO|rowE] at cols 0:48, corner at 72
            blk[:, 47, 1::2] = edge[0:64, 0:48]
            blk[:, 47, 0::2] = edge[64:128, 0:48]
            blk[:, 47, 95] = edge[0:64, 72]
        out[b, :, half * 48 : (half + 1) * 48, :] = blk
    return out



# revision 27
# speedup vs baseline: 1.0326x; 1.0054x over previous
"""Trainium2 Bass kernel for nn_NormConvTranspose2d.

Math: the reference applies, per (out-channel o, in-channel c), a
ConvTranspose2d(stride=2, k=3, pad=1, outpad=1) to input channel c with
kernel K[o,c], divides by the same convT applied to an all-ones image
(+eps), multiplies by weight[o,c], sums over c, adds bias.

With stride 2 / k 3, each output pixel (h', w') parity class is a fixed
1-4 tap correlation of the 48x48 input, and the "norm" denominator is a
per-(o,c) constant within each parity class (except at the last output
row/column).  So y/norm folds into effective channel-mixing matrices
W_tap[o,c] = weight*ktap/denom, and the whole module becomes channel-
mixing matmuls over column-shifted views of the input.

The host stacks x on 128 partitions as (x ; x shifted +48 = one row),
so every parity class needs only K=128 matmuls against column-shifted
views of ONE SBUF tile (one DMA, both halves at once).  With A=[oo|eo]
and B=[oe|ee] per chunk (8 output row-pairs, N=384), 3 matmuls total:
  A = [[Wi;Wc]|[Wf;0]] @ t[fb]  +  [[Wg;Wa]|[Wd;0]] @ t[fb+1]
  B = [[Wh;Wb]|[Wee;0]] @ t[fb]
Bias is applied by the PSUM->SBUF copy (per-partition bias operand);
the parity de-interleave happens on the host during the gather.
Edge passes (w'=95 col, h'=95 row, corner) are one small matmul each.

Sharding: 8 cores = 4 batches x 2 output-row halves (48 rows each).
No cross-core communication.
"""

import numpy as np

EPS = 1e-10
B, C, O, H, W = 4, 64, 64, 48, 48
HO = WO = 96
SLAB = 25          # input rows per core (24 + halo)
L = SLAB * 48      # 1200
LP = 1216          # padded free size of x tile
CHUNKS = (7, 6, 5, 6)   # row-pairs per chunk (big first: chunk 0 chews
                        # long enough that later chunks never stall on the
                        # input stream; 4 chunks, not 5: fewer tail
                        # LDWEIGHTS that can't hide under the tiny edge
                        # matmuls)
NMM = 480               # largest chunk's moving free size (PSUM tile width)
XSPLIT = 344            # first-slice columns of the x DMA (covers chunk 0;
                        # all slices stay >=512B/partition for DMA line rate)
WBW = 776              # wb width (bias col + 7 lhsT blocks)
WBSPLIT = 392          # main-chunk weights (bias+PA1+PA2+PB); splitting
                       # earlier delays the B matmuls and breaks the
                       # per-chunk A/B cadence
XOFF = WBSPLIT         # x's column offset inside the packed input tensor
XWW = WBSPLIT + LP + (WBW - WBSPLIT)   # 392 + 1216 + 384 = 1992
EOFF = WBSPLIT + LP    # edge-weight block offset (1608)
OUT_MAIN = 2 * 24 * 48     # 2304
OUT_W = OUT_MAIN + 73      # + rowEO(48) + colEO(24) + corner(1)

USE_BF16 = True    # bf16 datapath (f32 PSUM accumulate); else fp32r
USE_FP32R = True   # only relevant when USE_BF16 is False

_prog_cache = {}


def _build_program():
    import concourse.mybir as mybir
    import concourse.tile as tile
    from concourse import bacc

    f32 = mybir.dt.float32
    if USE_BF16:
        fmm = mybir.dt.bfloat16
    else:
        fmm = mybir.dt.float32r if USE_FP32R else f32
    Ident = mybir.ActivationFunctionType.Identity

    nc = bacc.Bacc("TRN2", target_bir_lowering=False, debug=False, num_devices=8)
    fio = fmm if USE_BF16 else f32
    # single packed input tensor [wb_main(392) | x(1200) | pad(16) | wb_edge]
    # so one DMA carries both of the first matmul's dependencies
    xw_d = nc.dram_tensor("xw", [128, XWW], fio, kind="ExternalInput").ap()
    out_d = nc.dram_tensor("out", [128, OUT_W], fio,
                           kind="ExternalOutput").ap()

    def D(ap):  # DRAM-side view matching the mm dtype tag
        return ap if USE_BF16 else ap.bitcast(fmm)

    with tile.TileContext(nc) as tc:
        with (
            tc.tile_pool(name="const", bufs=1) as cpool,
            tc.tile_pool(name="psum", bufs=3, space="PSUM") as ppool,
            tc.tile_pool(name="psum1", bufs=1, space="PSUM") as ppool1,
        ):
            xw = cpool.tile([128, XWW], fmm)
            och = cpool.tile([128, OUT_W], fio)
            warm = cpool.tile([64, 1], f32)

            nc.vector.memset(warm[:], 0.0)

            # input in 3 DMAs, ALL on the sync queue: the 16 DMA engines
            # process one queue's descriptors FIFO, so the slices land in
            # exact consumption order with no bandwidth competition — the
            # first slice (weights + chunk-0 x) gets the full line rate.
            # NOTE: slices must keep >=~1KB per partition row — small-row
            # DMAs run at a fraction of line rate (measured).
            S1 = XOFF + XSPLIT
            S2 = XOFF + 868
            nc.sync.dma_start(xw[:, 0:S1], D(xw_d[:, 0:S1]))
            nc.sync.dma_start(xw[:, S1:S2], D(xw_d[:, S1:S2]))
            nc.sync.dma_start(xw[:, S2:XWW], D(xw_d[:, S2:XWW]))
            # warm the Scalar activation table early
            nc.scalar.activation(warm[:], warm[:], Ident, bias=0.0)

            # single tiny PE wake-up matmul (zeroed scratch, result unread):
            # keeps the tensor engine sequencer from cold-starting when the
            # first input slice lands
            wz = cpool.tile([32, 160], fmm)
            nc.vector.memset(wz[:], 0.0)
            wp = ppool.tile([128, NMM], f32, tag="A")
            nc.tensor.matmul(wp[:, 0:32], wz[:, 0:128], wz[:, 128:160],
                             start=True, stop=True)

            if USE_BF16:
                # upcast the per-partition bias column once for the copies
                btile = cpool.tile([128, 1], f32)
                nc.vector.tensor_copy(btile[:], xw[:, 0:1])
                bt = btile[:]
            else:
                bt = xw[:, 0:1].bitcast(f32)

            def Wp(off, m=128):
                return xw[:, off : off + m]

            PA1, PA2, PB = Wp(8), Wp(136), Wp(264)
            R, R2 = Wp(EOFF), Wp(EOFF + 128, 64)
            Cm, CR = Wp(EOFF + 192), Wp(EOFF + 320, 64)

            def xs(fb, n=NMM):
                return xw[:, XOFF + fb : XOFF + fb + n]

            # stage the w'=95 input column for the edge matmuls (xcol
            # bottom = x[r+1, 47] via the +48-shifted stacked half, same
            # partitions).  On gpsimd: it's gated on the last input slice,
            # and on vector it would block the chunk A-adds (in-order queue)
            xvt = xw[0:64, XOFF : XOFF + L].rearrange("p (r q) -> p r q",
                                                      q=48)
            xvb = xw[64:128, XOFF : XOFF + L].rearrange("p (r q) -> p r q",
                                                        q=48)
            xcol = cpool.tile([128, 32], fmm)
            nc.gpsimd.tensor_copy(xcol[0:64, 0:SLAB], xvt[:, :, 47])
            nc.gpsimd.tensor_copy(xcol[64:128, 0:24], xvb[:, 0:24, 47])

            def chunk(ci):
                r0 = sum(CHUNKS[:ci])
                n = CHUNKS[ci] * 48
                fb, base = 48 * r0, 96 * r0
                A = ppool.tile([128, NMM], f32, tag="A")
                nc.tensor.matmul(A[:, 0:n], PA1, xs(fb, n), start=True,
                                 stop=False)
                nc.tensor.matmul(A[:, 0:n], PA2, xs(fb + 1, n), start=False,
                                 stop=True)
                Bp = ppool.tile([128, NMM], f32, tag="B")
                nc.tensor.matmul(Bp[:, 0:n], PB, xs(fb, n), start=True,
                                 stop=True)
                nc.vector.tensor_scalar_add(och[:, base : base + n],
                                            A[:, 0:n], bt)
                nc.scalar.activation(och[:, base + n : base + 2 * n],
                                     Bp[:, 0:n], Ident, bias=bt)

            chunk(0)
            chunk(1)
            # out DMAs grouped (fewer trips through the shared HWDGE
            # descriptor unit, ~625ns each); all on the idle sync queue,
            # staggered so the engine pool streams them back-to-back
            G0 = 96 * sum(CHUNKS[:2])
            nc.sync.dma_start(out_d[:, 0:G0], och[:, 0:G0])
            chunk(2)
            G1 = 96 * sum(CHUNKS[:3])
            nc.sync.dma_start(out_d[:, G0:G1], och[:, G0:G1])
            chunk(3)

            # --- edge passes (results appended at och[:, 2304:]) ---
            # scheduled last: they read the tail input slice.  All edge
            # results land in ONE PSUM bank so a single bias-add produces
            # the final och region (shortest possible last och write).
            # E[:,0:48] = last output row (h'=95; host uses it for half=1),
            # E[:,48:72] = last output column (w'=95), E[:,72] = corner.
            E = ppool1.tile([128, 80], f32, tag="E")
            nc.tensor.matmul(E[:, 0:48], R, xs(1104, 48), start=True,
                             stop=False)
            nc.tensor.matmul(E[0:64, 0:48], R2, xs(1105, 48), start=False,
                             stop=True)
            nc.tensor.matmul(E[:, 48:72], Cm, xcol[:, 0:24], start=True,
                             stop=True)
            # N=1 is illegal for fp32r; bf16 runs it padded to N=2 and the
            # fp32r path falls back to a plain-f32 matmul
            if USE_BF16:
                nc.tensor.matmul(E[0:64, 72:74], CR, xs(1151, 2),
                                 start=True, stop=True)
            else:
                nc.tensor.matmul(E[0:64, 72:73], CR.bitcast(f32),
                                 xs(1151, 1).bitcast(f32), start=True,
                                 stop=True)
            nc.vector.tensor_scalar_add(och[:, 2304:2377], E[:, 0:73], bt)

            # final group: last chunk + the edge columns
            nc.sync.dma_start(out_d[:, G1:OUT_W], och[:, G1:OUT_W])

    nc.compile()
    return nc


def _io_dtype():
    if USE_BF16:
        import ml_dtypes
        return ml_dtypes.bfloat16
    return np.float32


def _round_fp32r(a):
    """Quantize to the PE grid: bf16, or 11-mantissa-bit FP32R."""
    if USE_BF16:
        return np.ascontiguousarray(a, np.float32).astype(_io_dtype())
    if not USE_FP32R:
        return np.ascontiguousarray(a, np.float32)
    u = np.ascontiguousarray(a, np.float32).view(np.uint32)
    r = (u + np.uint32(0x7FF) + ((u >> np.uint32(12)) & np.uint32(1))) \
        & np.uint32(0xFFFFF000)
    return r.view(np.float32)


def _eff_weights(weight, kernels, bias):
    """Host-side constant folding: effective channel-mix matrices packed as
    K=128 lhsT blocks [128, 712] (col 0 = per-partition bias for the copy
    ops; the K rows match the stacked (x ; x+1) moving tile)."""
    w = weight.astype(np.float64)
    k = kernels.astype(np.float64)
    k00, k01, k02 = k[:, :, 0, 0], k[:, :, 0, 1], k[:, :, 0, 2]
    k10, k11, k12 = k[:, :, 1, 0], k[:, :, 1, 1], k[:, :, 1, 2]
    k20, k21, k22 = k[:, :, 2, 0], k[:, :, 2, 1], k[:, :, 2, 2]

    den_oo = k22 + k20 + k02 + k00 + EPS
    M = dict(
        Wee=w * k11 / (k11 + EPS),
        Wf=w * k12 / (k12 + k10 + EPS), Wd=w * k10 / (k12 + k10 + EPS),
        Wh=w * k21 / (k21 + k01 + EPS), Wb=w * k01 / (k21 + k01 + EPS),
        Wi=w * k22 / den_oo, Wg=w * k20 / den_oo,
        Wc=w * k02 / den_oo, Wa=w * k00 / den_oo,
        Ef=w * k12 / (k12 + EPS),
        Ei=w * k22 / (k22 + k02 + EPS), Ec=w * k02 / (k22 + k02 + EPS),
        Rh=w * k21 / (k21 + EPS),
        Ri=w * k22 / (k22 + k20 + EPS), Rg=w * k20 / (k22 + k20 + EPS),
        Ci=w * k22 / (k22 + EPS),
    )
    T = {n: m.T for n, m in M.items()}  # lhsT orientation [c, o]
    Z = np.zeros((64, 64))

    wbm = np.zeros((128, WBW))
    wbm[0:64, 0] = bias.astype(np.float64)
    wbm[64:128, 0] = bias.astype(np.float64)
    blocks = [
        (8, [[T["Wi"], T["Wf"]], [T["Wc"], Z]]),          # PA1 @ t[fb]
        (136, [[T["Wg"], T["Wd"]], [T["Wa"], Z]]),        # PA2 @ t[fb+1]
        (264, [[T["Wh"], T["Wee"]], [T["Wb"], Z]]),       # PB  @ t[fb]
        (392, [[T["Ri"], T["Rh"]], [Z, Z]]),              # R   @ t[1104]
        (520, [[T["Rg"]], [Z]]),                          # R2  @ t[1105]
        (584, [[T["Ei"], T["Ef"]], [T["Ec"], Z]]),        # Cm  @ xcol
        (712, [[T["Ci"]], [Z]]),                          # CR  @ t[1151]
    ]
    for off, blk in blocks:
        b = np.block(blk)
        wbm[:, off : off + b.shape[1]] = b
    return _round_fp32r(wbm)


def _make_in_maps(input, weight, kernels, bias):
    dt = _io_dtype()
    wbm = _eff_weights(weight, kernels, bias)
    x = np.asarray(input, np.float32).astype(dt)
    in_maps = []
    for core in range(8):
        b, half = core // 2, core % 2
        slab = np.zeros((C, SLAB, 48), dt)
        if half == 0:
            slab[:, :, :] = x[b, :, 0:25, :]
        else:
            slab[:, 0:24, :] = x[b, :, 24:48, :]
        flat = slab.reshape(C, L)
        xw = np.zeros((128, XWW), dt)
        xw[:, 0:WBSPLIT] = wbm[:, 0:WBSPLIT]
        xw[0:64, XOFF : XOFF + L] = flat
        xw[64:128, XOFF : XOFF + L - 48] = flat[:, 48:]
        xw[:, EOFF:XWW] = wbm[:, WBSPLIT:WBW]
        in_maps.append({"xw": xw})
    return in_maps


def kernel(input, weight, kernels, bias):
    from concourse.bass_utils import run_bass_kernel_spmd

    input = np.asarray(input)
    weight = np.asarray(weight)
    kernels = np.asarray(kernels)
    bias = np.asarray(bias)

    if "nc" not in _prog_cache:
        _prog_cache["nc"] = _build_program()
    nc = _prog_cache["nc"]

    in_maps = _make_in_maps(input, weight, kernels, bias)
    res = run_bass_kernel_spmd(nc, in_maps, core_ids=list(range(8)))

    out = np.empty((B, O, HO, WO), np.float32)
    blk = np.empty((O, 48, WO), np.float32)
    for core in range(8):
        b, half = core // 2, core % 2
        r = np.asarray(res.results[core]["out"]).astype(np.float32)
        edge = r[:, OUT_MAIN:]
        # per chunk: [A(n) | B(n)] with A=[oo|eo], B=[oe|ee]
        r0 = 0
        for nc_ in CHUNKS:
            base, n = 96 * r0, 48 * nc_
            Ab = r[:, base : base + n].reshape(128, nc_, 48)
            Bb = r[:, base + n : base + 2 * n].reshape(128, nc_, 48)
            rows = slice(2 * r0, 2 * (r0 + nc_))
            blk[:, rows, :][:, 1::2, 1::2] = Ab[0:64]
            blk[:, rows, :][:, 0::2, 1::2] = Ab[64:128]
            blk[:, rows, :][:, 1::2, 0::2] = Bb[0:64]
            blk[:, rows, :][:, 0::2, 0::2] = Bb[64:128]
            r0 += nc_
        # w'=95 column fixup: Ec=[colO|colE] at cols 48:72
        blk[:, 1::2, 95] = edge[0:64, 48:72]
        blk[:, 0::2, 95] = edge[64:128, 48:72]
        if half == 1:
            # h'=95 row fixup: Er=[rowO|rowE] at cols 0:48, corner at 72
            blk[:, 47, 1::2] = edge[0:64, 0:48]
            blk[:, 47, 0::2] = edge[64:128, 0:48]
            blk[:, 47, 95] = edge[0:64, 72]
        out[b, :, half * 48 : (half + 1) * 48, :] = blk
    return out

